# revision 30
# baseline (speedup 1.0000x reference)
"""Trainium2 Bass kernel for the coverage-attention module.

Computes, for B=16384, K=5, H=E=512:
    proj_keys = einsum("bke,he->bkh", topics, Ua)
    q_i = query @ (Wa^T)^i            (i = 1..K)
    s_i = tanh(q_i + proj_keys[:,i]) @ va_w^T + va_b
    scores[:, i] = s_i * coverage[:, i]
    alphas = softmax(scores, axis=1)
    mt = einsum("bk,bke->be", alphas, topics)
Returns (mt, alphas) like the reference.

Strategy: data-parallel over batch across 8 NeuronCores (2048 rows/core).
Host precomputes W_i = (Wa^T)^i in float64 and ships [W_1..W_K] so the
query chain becomes one matmul; both matmuls accumulate into the same
PSUM bank so the q+proj_key add is free.  All matmul operands are bf16
(host-cast); accumulation and the softmax/weighted-sum epilogue are fp32.
"""

import os
import sys

import numpy as np

try:
    import ml_dtypes
except ImportError:  # pragma: no cover
    sys.path.insert(0, "/opt/trn_rl_repo")
    import ml_dtypes

for _p in ("/opt/trn_rl_repo",):
    if _p not in sys.path and os.path.isdir(_p):
        sys.path.insert(0, _p)

BF16 = ml_dtypes.bfloat16
FP8 = ml_dtypes.float8_e4m3

B, K, H, E = 16384, 5, 512, 512
NCORES = 8
BS = B // NCORES          # rows per core = 2048
NT = BS // 128            # 128-row tiles per core = 16
HC = H // 128             # contraction chunks over h = 4
EC = E // 128             # contraction chunks over e = 4

_CACHE = {}
LAST = {}                 # exec_time_ns etc. from the most recent run
RUN_KWARGS = {}           # extra kwargs for run_bass_kernel_spmd (test harness)


def _build_nc():
    import concourse.bass as bass
    import concourse.mybir as mybir
    import concourse.tile as tile
    from concourse import bacc
    from concourse.bass import ts

    f32 = mybir.dt.float32
    bf16 = mybir.dt.bfloat16
    f8 = mybir.dt.float8e4
    DR = mybir.MatmulPerfMode.DoubleRow
    ALU = mybir.AluOpType
    ACTF = mybir.ActivationFunctionType
    AX = mybir.AxisListType

    nc = bacc.Bacc()

    qT_d = nc.declare_dram_parameter("qT", [128, HC, BS], bf16, isOutput=False)
    W_d = nc.declare_dram_parameter("Wstack", [128, HC, K * H], bf16, isOutput=False)
    # fp8 DoubleRow operands: [.., 2(pair), 2(row-in-pair), ..]
    UaT_d = nc.declare_dram_parameter("UaT", [128, 2, 2, H], f8, isOutput=False)
    tT_d = nc.declare_dram_parameter(
        "topicsT", [NT, 128, K, 2, 2, 128], f8, isOutput=False
    )
    tN_d = nc.declare_dram_parameter("topicsN", [NT, 128, K, E], bf16, isOutput=False)
    cov_d = nc.declare_dram_parameter("cov", [128, NT, K], f32, isOutput=False)
    va_d = nc.declare_dram_parameter("va", [128, H], bf16, isOutput=False)
    vab_d = nc.declare_dram_parameter("vab", [128, 1], f32, isOutput=False)

    mt_d = nc.declare_dram_parameter("mt", [NT, 128, E], bf16, isOutput=True)
    al_d = nc.declare_dram_parameter("alphas", [128, NT, K], f32, isOutput=True)

    with tile.TileContext(nc) as tc:
        with (
            tc.tile_pool(name="const", bufs=1) as cpool,
            tc.tile_pool(name="stream", bufs=3) as spool,
            tc.tile_pool(name="work", bufs=2) as wpool,
            tc.tile_pool(name="tanh", bufs=8) as tpool,
            tc.tile_pool(name="psum", bufs=1, space=bass.MemorySpace.PSUM) as ppool,
        ):
            UaT = cpool.tile([128, 2, 2, H], f8, tag="UaT")
            va = cpool.tile([128, H], bf16, tag="va")
            vab = cpool.tile([128, 1], f32, tag="vab")
            cov = cpool.tile([128, NT, K], f32, tag="cov")
            Wst = cpool.tile([128, HC, K * H], bf16, tag="Wst")
            alph = cpool.tile([128, NT, K], f32, tag="alph")

            # Small consts first so the first PK matmul unblocks fast;
            # Wstack is only needed once the first QW pass starts.
            # Two HWDGE rings: sync carries the matmul-critical stationaries
            # (UaT, topicsT, qT); scalar carries consts + epilogue traffic
            # (va, cov, Wstack, topicsN, outputs) so tile-0's stationaries
            # aren't queued behind megabytes of lower-priority transfers.
            nc.sync.dma_start(out=UaT[:], in_=UaT_d[:])
            nc.scalar.dma_start(out=va[:], in_=va_d[:])
            nc.scalar.dma_start(out=vab[:], in_=vab_d[:])
            nc.scalar.dma_start(out=cov[:], in_=cov_d[:])
            for k in range(K):
                nc.scalar.dma_start(
                    out=Wst[:, :, ts(k, 512)], in_=W_d[:, :, ts(k, 512)]
                )

            for t in range(NT):
                tT = spool.tile([128, K, 2, 2, 128], f8, tag="tT")
                nc.sync.dma_start(out=tT[:], in_=tT_d[t])
                qT = spool.tile([128, HC, 128], bf16, tag="qT")
                nc.sync.dma_start(out=qT[:], in_=qT_d[:, :, ts(t, 128)])

                ps = [
                    ppool.tile([128, 512], f32, tag=f"ps{k}", name=f"ps{k}")
                    for k in range(K)
                ]

                scores = wpool.tile([128, K], f32, tag="scores")
                # All PK matmuls first (need only tT+UaT — cheap DMA deps),
                # then QW k-outer so bank k completes staggered for tanh.
                for k in range(K):
                    for c2 in range(2):
                        nc.tensor.matmul(
                            ps[k][:],
                            tT[:, k, c2, :, :],
                            UaT[:, c2, :, :],
                            start=(c2 == 0),
                            stop=False,
                            perf_mode=DR,
                        )
                for k in range(K):
                    for c in range(HC):
                        nc.tensor.matmul(
                            ps[k][:],
                            qT[:, c, :],
                            Wst[:, c, ts(k, 512)],
                            start=False,
                            stop=(c == HC - 1),
                        )
                    th = tpool.tile([128, 512], bf16, tag="tanh")
                    nc.scalar.activation(th[:], ps[k][:], ACTF.Tanh)
                    scr = wpool.tile([128, 512], bf16, tag="ttr")
                    # scr = th * va; scores[:,k] = sum(scr)
                    nc.vector.scalar_tensor_tensor(
                        out=scr[:],
                        in0=th[:],
                        scalar=1.0,
                        in1=va[:],
                        op0=ALU.bypass,
                        op1=ALU.mult,
                        accum_out=scores[:, k : k + 1],
                    )

                # topics in natural layout — needed only by the mt epilogue,
                # so its DMA is emitted late (lower queue priority).
                tN = spool.tile([128, K, E], bf16, tag="tN")
                nc.scalar.dma_start(out=tN[:], in_=tN_d[t])

                # softmax over K: sc2 = (scores + va_b) * coverage
                sc2 = wpool.tile([128, K], f32, tag="sc2")
                nc.vector.scalar_tensor_tensor(
                    out=sc2[:],
                    in0=scores[:],
                    scalar=vab[:, 0:1],
                    in1=cov[:, t, :],
                    op0=ALU.add,
                    op1=ALU.mult,
                )
                negmax = wpool.tile([128, 1], f32, tag="negmax")
                nc.vector.reduce_max(negmax[:], sc2[:], axis=AX.X, negate=True)
                exps = wpool.tile([128, K], f32, tag="exps")
                nc.scalar.activation(
                    exps[:], sc2[:], ACTF.Exp, bias=negmax[:, 0:1], scale=1.0
                )
                ssum = wpool.tile([128, 1], f32, tag="ssum")
                nc.vector.reduce_sum(ssum[:], exps[:], axis=AX.X)
                rsum = wpool.tile([128, 1], f32, tag="rsum")
                nc.vector.reciprocal(rsum[:], ssum[:])
                nc.vector.tensor_scalar(
                    alph[:, t, :], exps[:], rsum[:, 0:1], None, ALU.mult
                )

                # mt = sum_k alpha_k * topics_k: bf16 tensor_scalar (4x mode)
                # then a bf16 add tree (2x mode)
                sk = []
                for k in range(K):
                    s = wpool.tile([128, E], bf16, tag=f"mts{k}", name=f"mts{k}")
                    nc.vector.tensor_scalar(
                        s[:], tN[:, k, :], alph[:, t, k : k + 1], None, ALU.mult
                    )
                    sk.append(s)
                # add tree on the otherwise-idle GpSimd engine
                a01 = wpool.tile([128, E], bf16, tag="a01")
                nc.gpsimd.tensor_tensor(a01[:], sk[0][:], sk[1][:], ALU.add)
                a23 = wpool.tile([128, E], bf16, tag="a23")
                nc.gpsimd.tensor_tensor(a23[:], sk[2][:], sk[3][:], ALU.add)
                a014 = wpool.tile([128, E], bf16, tag="a014")
                nc.gpsimd.tensor_tensor(a014[:], a01[:], sk[4][:], ALU.add)
                mt = wpool.tile([128, E], bf16, tag="mt")
                nc.gpsimd.tensor_tensor(mt[:], a014[:], a23[:], ALU.add)
                nc.scalar.dma_start(out=mt_d[t], in_=mt[:])

            nc.scalar.dma_start(out=al_d[:], in_=alph[:])

    nc.finalize()
    return nc


def _prep_core_inputs(query, topics, coverage_vector, Ua, Wa, va_w, va_b):
    """Build the 8 per-core input maps (all host-side layout/dtype work)."""
    # W_i = (Wa^T)^i, i=1..K, computed in float64 then cast once to bf16.
    WaT = Wa.astype(np.float64).T
    mats = []
    cur = np.eye(H, dtype=np.float64)
    for _ in range(K):
        cur = cur @ WaT
        mats.append(cur)
    Wstack = np.concatenate(mats, axis=1)  # (H, K*H)
    W_sb = np.ascontiguousarray(
        Wstack.reshape(HC, 128, K * H).transpose(1, 0, 2)
    ).astype(BF16)

    # DoubleRow fp8 layout for Ua^T: [e_in, pair(c2), row(i), h],
    # e = c2*256 + i*128 + e_in
    UaT_sb = np.ascontiguousarray(
        Ua.T.reshape(2, 2, 128, H).transpose(2, 0, 1, 3)
    ).astype(FP8)
    va_sb = np.ascontiguousarray(np.broadcast_to(va_w.astype(BF16), (128, H)))
    vab_sb = np.full((128, 1), np.float32(va_b[0]), np.float32)

    tb = topics.astype(BF16)          # (B, K, E)
    t8 = topics.astype(FP8)           # (B, K, E) for the PK matmul
    qb = query.astype(BF16)           # (B, H)
    cova = np.asarray(coverage_vector, np.float32)

    in_maps = []
    for c in range(NCORES):
        sl = slice(c * BS, (c + 1) * BS)
        tsh = tb[sl]                                       # (BS, K, E)
        tN = np.ascontiguousarray(tsh.reshape(NT, 128, K, E))
        # [t, e_in, k, c2, i, b] with e = c2*256 + i*128 + e_in
        tT = np.ascontiguousarray(
            t8[sl].reshape(NT, 128, K, 2, 2, 128).transpose(0, 5, 2, 3, 4, 1)
        )
        qT = np.ascontiguousarray(
            qb[sl].T.reshape(HC, 128, BS).transpose(1, 0, 2)
        )
        cv = np.ascontiguousarray(cova[sl].reshape(NT, 128, K).transpose(1, 0, 2))
        in_maps.append(
            {
                "qT": qT,
                "Wstack": W_sb,
                "UaT": UaT_sb,
                "topicsT": tT,
                "topicsN": tN,
                "cov": cv,
                "va": va_sb,
                "vab": vab_sb,
            }
        )
    return in_maps


def kernel(query, topics, coverage_vector, Ua, Wa, va_w, va_b):
    from concourse.bass_utils import run_bass_kernel_spmd

    query = np.asarray(query, np.float32)
    topics = np.asarray(topics, np.float32)
    coverage_vector = np.asarray(coverage_vector, np.float32)
    Ua = np.asarray(Ua, np.float32)
    Wa = np.asarray(Wa, np.float32)
    va_w = np.asarray(va_w, np.float32)
    va_b = np.asarray(va_b, np.float32)

    if "nc" not in _CACHE:
        _CACHE["nc"] = _build_nc()
    nc = _CACHE["nc"]

    in_maps = _prep_core_inputs(
        query, topics, coverage_vector, Ua, Wa, va_w, va_b
    )

    res = run_bass_kernel_spmd(
        nc, in_maps, core_ids=list(range(NCORES)), **RUN_KWARGS
    )
    LAST["exec_time_ns"] = res.exec_time_ns
    LAST["results"] = res

    mt = np.concatenate(
        [
            res.results[c]["mt"].astype(np.float32).reshape(BS, E)
            for c in range(NCORES)
        ],
        axis=0,
    )
    alphas = np.concatenate(
        [
            res.results[c]["alphas"].transpose(1, 0, 2).reshape(BS, K)
            for c in range(NCORES)
        ],
        axis=0,
    ).astype(np.float32)
    return (mt, alphas)


# revision 39
# speedup vs baseline: 1.6669x; 1.6669x over previous
"""Trainium2 Bass kernel for the coverage-attention module.

Computes, for B=16384, K=5, H=E=512:
    proj_keys = einsum("bke,he->bkh", topics, Ua)
    q_i = query @ (Wa^T)^i            (i = 1..K)
    s_i = tanh(q_i + proj_keys[:,i]) @ va_w^T + va_b
    scores[:, i] = s_i * coverage[:, i]
    alphas = softmax(scores, axis=1)
    mt = einsum("bk,bke->be", alphas, topics)
Returns (mt, alphas) like the reference.

Strategy: data-parallel over batch across 8 NeuronCores (2048 rows/core).
Host precomputes W_i = (Wa^T)^i in float64 and ships [W_1..W_K] so the
query chain becomes one matmul; both matmuls accumulate into the same
PSUM bank so the q+proj_key add is free.  All matmul operands are bf16
(host-cast); accumulation and the softmax/weighted-sum epilogue are fp32.
"""

import os
import sys

import numpy as np

try:
    import ml_dtypes
except ImportError:  # pragma: no cover
    sys.path.insert(0, "/opt/trn_rl_repo")
    import ml_dtypes

for _p in ("/opt/trn_rl_repo",):
    if _p not in sys.path and os.path.isdir(_p):
        sys.path.insert(0, _p)

BF16 = ml_dtypes.bfloat16
FP8 = ml_dtypes.float8_e4m3

B, K, H, E = 16384, 5, 512, 512
NCORES = 8
BS = B // NCORES          # rows per core = 2048
NT = BS // 128            # 128-row tiles per core = 16
HC = H // 128             # contraction chunks over h = 4
EC = E // 128             # contraction chunks over e = 4

_CACHE = {}
LAST = {}                 # exec_time_ns etc. from the most recent run
RUN_KWARGS = {}           # extra kwargs for run_bass_kernel_spmd (test harness)


def _build_nc():
    import concourse.bass as bass
    import concourse.mybir as mybir
    import concourse.tile as tile
    from concourse import bacc
    from concourse.bass import ts

    f32 = mybir.dt.float32
    bf16 = mybir.dt.bfloat16
    f8 = mybir.dt.float8e4
    DR = mybir.MatmulPerfMode.DoubleRow
    ALU = mybir.AluOpType
    ACTF = mybir.ActivationFunctionType
    AX = mybir.AxisListType

    nc = bacc.Bacc()

    qT_d = nc.declare_dram_parameter("qT", [128, 2, 2, BS], f8, isOutput=False)
    W_d = nc.declare_dram_parameter("Wstack", [128, 2, 2, K * H], f8, isOutput=False)
    # fp8 DoubleRow operands: [.., 2(pair), 2(row-in-pair), ..]
    UaT_d = nc.declare_dram_parameter("UaT", [128, 2, 2, H], f8, isOutput=False)
    tT_d = nc.declare_dram_parameter(
        "topicsT", [NT, 128, K, 2, 2, 128], f8, isOutput=False
    )
    tN_d = nc.declare_dram_parameter("topicsN", [NT, 128, K, E], bf16, isOutput=False)
    cov_d = nc.declare_dram_parameter("cov", [128, NT, K], f32, isOutput=False)
    va_d = nc.declare_dram_parameter("va", [128, H], bf16, isOutput=False)
    vab_d = nc.declare_dram_parameter("vab", [128, 1], f32, isOutput=False)

    mt_d = nc.declare_dram_parameter("mt", [NT, 128, E], bf16, isOutput=True)
    al_d = nc.declare_dram_parameter("alphas", [128, NT, K], f32, isOutput=True)

    with tile.TileContext(nc) as tc:
        with (
            tc.tile_pool(name="const", bufs=1) as cpool,
            tc.tile_pool(name="stream", bufs=3) as spool,
            tc.tile_pool(name="work", bufs=2) as wpool,
            tc.tile_pool(name="tanh", bufs=8) as tpool,
            tc.tile_pool(name="psum", bufs=1, space=bass.MemorySpace.PSUM) as ppool,
        ):
            UaT = cpool.tile([128, 2, 2, H], f8, tag="UaT")
            va = cpool.tile([128, H], bf16, tag="va")
            vab = cpool.tile([128, 1], f32, tag="vab")
            cov = cpool.tile([128, NT, K], f32, tag="cov")
            Wst = cpool.tile([128, 2, 2, K * H], f8, tag="Wst")
            alph = cpool.tile([128, NT, K], f32, tag="alph")

            # Small consts first so the first PK matmul unblocks fast;
            # Wstack is only needed once the first QW pass starts.
            # Two HWDGE rings: sync carries the matmul-critical stationaries
            # (UaT, topicsT, qT); scalar carries consts + epilogue traffic
            # (va, cov, Wstack, topicsN, outputs) so tile-0's stationaries
            # aren't queued behind megabytes of lower-priority transfers.
            nc.sync.dma_start(out=UaT[:], in_=UaT_d[:])
            nc.scalar.dma_start(out=va[:], in_=va_d[:])
            nc.scalar.dma_start(out=vab[:], in_=vab_d[:])
            nc.scalar.dma_start(out=cov[:], in_=cov_d[:])
            for k in range(K):
                nc.scalar.dma_start(
                    out=Wst[:, :, :, ts(k, 512)], in_=W_d[:, :, :, ts(k, 512)]
                )

            for t in range(NT):
                tT = spool.tile([128, K, 2, 2, 128], f8, tag="tT")
                nc.sync.dma_start(out=tT[:], in_=tT_d[t])
                qT = spool.tile([128, 2, 2, 128], f8, tag="qT")
                nc.sync.dma_start(out=qT[:], in_=qT_d[:, :, :, ts(t, 128)])

                ps = [
                    ppool.tile([128, 512], f32, tag=f"ps{k}", name=f"ps{k}")
                    for k in range(K)
                ]

                scores = wpool.tile([128, K], f32, tag="scores")
                # All PK matmuls first (need only tT+UaT — cheap DMA deps),
                # then QW k-outer so bank k completes staggered for tanh.
                for k in range(K):
                    for c2 in range(2):
                        nc.tensor.matmul(
                            ps[k][:],
                            tT[:, k, c2, :, :],
                            UaT[:, c2, :, :],
                            start=(c2 == 0),
                            stop=False,
                            perf_mode=DR,
                        )
                for k in range(K):
                    for c2 in range(2):
                        nc.tensor.matmul(
                            ps[k][:],
                            qT[:, c2, :, :],
                            Wst[:, c2, :, ts(k, 512)],
                            start=False,
                            stop=(c2 == 1),
                            perf_mode=DR,
                        )
                    th = tpool.tile([128, 512], bf16, tag="tanh")
                    nc.scalar.activation(th[:], ps[k][:], ACTF.Tanh)
                    scr = wpool.tile([128, 512], bf16, tag="ttr")
                    # scr = th * va; scores[:,k] = sum(scr)
                    nc.vector.scalar_tensor_tensor(
                        out=scr[:],
                        in0=th[:],
                        scalar=1.0,
                        in1=va[:],
                        op0=ALU.bypass,
                        op1=ALU.mult,
                        accum_out=scores[:, k : k + 1],
                    )

                # topics in natural layout — needed only by the mt epilogue,
                # so its DMA is emitted late (lower queue priority).
                tN = spool.tile([128, K, E], bf16, tag="tN")
                nc.scalar.dma_start(out=tN[:], in_=tN_d[t])

                # softmax over K: sc2 = (scores + va_b) * coverage
                sc2 = wpool.tile([128, K], f32, tag="sc2")
                nc.vector.scalar_tensor_tensor(
                    out=sc2[:],
                    in0=scores[:],
                    scalar=vab[:, 0:1],
                    in1=cov[:, t, :],
                    op0=ALU.add,
                    op1=ALU.mult,
                )
                negmax = wpool.tile([128, 1], f32, tag="negmax")
                nc.vector.reduce_max(negmax[:], sc2[:], axis=AX.X, negate=True)
                exps = wpool.tile([128, K], f32, tag="exps")
                nc.scalar.activation(
                    exps[:], sc2[:], ACTF.Exp, bias=negmax[:, 0:1], scale=1.0
                )
                ssum = wpool.tile([128, 1], f32, tag="ssum")
                nc.vector.reduce_sum(ssum[:], exps[:], axis=AX.X)
                rsum = wpool.tile([128, 1], f32, tag="rsum")
                nc.vector.reciprocal(rsum[:], ssum[:])
                nc.vector.tensor_scalar(
                    alph[:, t, :], exps[:], rsum[:, 0:1], None, ALU.mult
                )

                # mt = sum_k alpha_k * topics_k: bf16 tensor_scalar (4x mode)
                # then a bf16 add tree (2x mode)
                sk = []
                for k in range(K):
                    s = wpool.tile([128, E], bf16, tag=f"mts{k}", name=f"mts{k}")
                    nc.vector.tensor_scalar(
                        s[:], tN[:, k, :], alph[:, t, k : k + 1], None, ALU.mult
                    )
                    sk.append(s)
                a01 = wpool.tile([128, E], bf16, tag="a01")
                nc.vector.tensor_tensor(a01[:], sk[0][:], sk[1][:], ALU.add)
                a23 = wpool.tile([128, E], bf16, tag="a23")
                nc.vector.tensor_tensor(a23[:], sk[2][:], sk[3][:], ALU.add)
                a014 = wpool.tile([128, E], bf16, tag="a014")
                nc.vector.tensor_tensor(a014[:], a01[:], sk[4][:], ALU.add)
                mt = wpool.tile([128, E], bf16, tag="mt")
                nc.vector.tensor_tensor(mt[:], a014[:], a23[:], ALU.add)
                nc.scalar.dma_start(out=mt_d[t], in_=mt[:])

            nc.scalar.dma_start(out=al_d[:], in_=alph[:])

    nc.finalize()
    return nc


def _prep_core_inputs(query, topics, coverage_vector, Ua, Wa, va_w, va_b):
    """Build the 8 per-core input maps (all host-side layout/dtype work)."""
    # W_i = (Wa^T)^i, i=1..K, computed in float64 then cast once to bf16.
    WaT = Wa.astype(np.float64).T
    mats = []
    cur = np.eye(H, dtype=np.float64)
    for _ in range(K):
        cur = cur @ WaT
        mats.append(cur)
    Wstack = np.concatenate(mats, axis=1)  # (H, K*H)
    # DR layout [h_in, c2, i, col], h = c2*256 + i*128 + h_in
    W_sb = np.ascontiguousarray(
        Wstack.reshape(2, 2, 128, K * H).transpose(2, 0, 1, 3)
    ).astype(FP8)

    # DoubleRow fp8 layout for Ua^T: [e_in, pair(c2), row(i), h],
    # e = c2*256 + i*128 + e_in
    UaT_sb = np.ascontiguousarray(
        Ua.T.reshape(2, 2, 128, H).transpose(2, 0, 1, 3)
    ).astype(FP8)
    va_sb = np.ascontiguousarray(np.broadcast_to(va_w.astype(BF16), (128, H)))
    vab_sb = np.full((128, 1), np.float32(va_b[0]), np.float32)

    tb = topics.astype(BF16)          # (B, K, E)
    t8 = topics.astype(FP8)           # (B, K, E) for the PK matmul
    q8 = query.astype(FP8)            # (B, H) for the QW matmul
    cova = np.asarray(coverage_vector, np.float32)

    in_maps = []
    for c in range(NCORES):
        sl = slice(c * BS, (c + 1) * BS)
        tsh = tb[sl]                                       # (BS, K, E)
        tN = np.ascontiguousarray(tsh.reshape(NT, 128, K, E))
        # [t, e_in, k, c2, i, b] with e = c2*256 + i*128 + e_in
        tT = np.ascontiguousarray(
            t8[sl].reshape(NT, 128, K, 2, 2, 128).transpose(0, 5, 2, 3, 4, 1)
        )
        qT = np.ascontiguousarray(
            q8[sl].T.reshape(2, 2, 128, BS).transpose(2, 0, 1, 3)
        )
        cv = np.ascontiguousarray(cova[sl].reshape(NT, 128, K).transpose(1, 0, 2))
        in_maps.append(
            {
                "qT": qT,
                "Wstack": W_sb,
                "UaT": UaT_sb,
                "topicsT": tT,
                "topicsN": tN,
                "cov": cv,
                "va": va_sb,
                "vab": vab_sb,
            }
        )
    return in_maps


def kernel(query, topics, coverage_vector, Ua, Wa, va_w, va_b):
    from concourse.bass_utils import run_bass_kernel_spmd

    query = np.asarray(query, np.float32)
    topics = np.asarray(topics, np.float32)
    coverage_vector = np.asarray(coverage_vector, np.float32)
    Ua = np.asarray(Ua, np.float32)
    Wa = np.asarray(Wa, np.float32)
    va_w = np.asarray(va_w, np.float32)
    va_b = np.asarray(va_b, np.float32)

    if "nc" not in _CACHE:
        _CACHE["nc"] = _build_nc()
    nc = _CACHE["nc"]

    in_maps = _prep_core_inputs(
        query, topics, coverage_vector, Ua, Wa, va_w, va_b
    )

    res = run_bass_kernel_spmd(
        nc, in_maps, core_ids=list(range(NCORES)), **RUN_KWARGS
    )
    LAST["exec_time_ns"] = res.exec_time_ns
    LAST["results"] = res

    mt = np.concatenate(
        [
            res.results[c]["mt"].astype(np.float32).reshape(BS, E)
            for c in range(NCORES)
        ],
        axis=0,
    )
    alphas = np.concatenate(
        [
            res.results[c]["alphas"].transpose(1, 0, 2).reshape(BS, K)
            for c in range(NCORES)
        ],
        axis=0,
    ).astype(np.float32)
    return (mt, alphas)


# revision 44
# speedup vs baseline: 1.7375x; 1.0423x over previous
"""Trainium2 Bass kernel for the coverage-attention module.

Computes, for B=16384, K=5, H=E=512:
    proj_keys = einsum("bke,he->bkh", topics, Ua)
    q_i = query @ (Wa^T)^i            (i = 1..K)
    s_i = tanh(q_i + proj_keys[:,i]) @ va_w^T + va_b
    scores[:, i] = s_i * coverage[:, i]
    alphas = softmax(scores, axis=1)
    mt = einsum("bk,bke->be", alphas, topics)
Returns (mt, alphas) like the reference.

Strategy: data-parallel over batch across 8 NeuronCores (2048 rows/core).
Host precomputes W_i = (Wa^T)^i in float64 and ships [W_1..W_K] so the
query chain becomes one matmul; both matmuls accumulate into the same
PSUM bank so the q+proj_key add is free.  All matmul operands are bf16
(host-cast); accumulation and the softmax/weighted-sum epilogue are fp32.
"""

import os
import sys

import numpy as np

try:
    import ml_dtypes
except ImportError:  # pragma: no cover
    sys.path.insert(0, "/opt/trn_rl_repo")
    import ml_dtypes

for _p in ("/opt/trn_rl_repo",):
    if _p not in sys.path and os.path.isdir(_p):
        sys.path.insert(0, _p)

BF16 = ml_dtypes.bfloat16
FP8 = ml_dtypes.float8_e4m3

B, K, H, E = 16384, 5, 512, 512
NCORES = 8
BS = B // NCORES          # rows per core = 2048
NT = BS // 128            # 128-row tiles per core = 16
HC = H // 128             # contraction chunks over h = 4
EC = E // 128             # contraction chunks over e = 4

_CACHE = {}
LAST = {}                 # exec_time_ns etc. from the most recent run
RUN_KWARGS = {}           # extra kwargs for run_bass_kernel_spmd (test harness)


def _build_nc():
    import concourse.bass as bass
    import concourse.mybir as mybir
    import concourse.tile as tile
    from concourse import bacc
    from concourse.bass import ts

    f32 = mybir.dt.float32
    bf16 = mybir.dt.bfloat16
    f8 = mybir.dt.float8e4
    DR = mybir.MatmulPerfMode.DoubleRow
    ALU = mybir.AluOpType
    ACTF = mybir.ActivationFunctionType
    AX = mybir.AxisListType

    nc = bacc.Bacc()

    qT_d = nc.declare_dram_parameter("qT", [128, 2, 2, BS], f8, isOutput=False)
    W_d = nc.declare_dram_parameter("Wstack", [128, 2, 2, K * H], f8, isOutput=False)
    # fp8 DoubleRow operands: [.., 2(pair), 2(row-in-pair), ..]
    UaT_d = nc.declare_dram_parameter("UaT", [128, 2, 2, H], f8, isOutput=False)
    tT_d = nc.declare_dram_parameter(
        "topicsT", [NT, 128, K, 2, 2, 128], f8, isOutput=False
    )
    tN_d = nc.declare_dram_parameter("topicsN", [NT, 128, K, E], bf16, isOutput=False)
    cov_d = nc.declare_dram_parameter("cov", [128, NT, K], f32, isOutput=False)
    va_d = nc.declare_dram_parameter("va", [128, H], bf16, isOutput=False)
    vab_d = nc.declare_dram_parameter("vab", [128, 1], f32, isOutput=False)
    id_d = nc.declare_dram_parameter("ident", [128, 128], bf16, isOutput=False)

    mt_d = nc.declare_dram_parameter("mt", [NT, 128, E], bf16, isOutput=True)
    al_d = nc.declare_dram_parameter("alphas", [128, NT, K], f32, isOutput=True)

    with tile.TileContext(nc) as tc:
        with (
            tc.tile_pool(name="const", bufs=1) as cpool,
            tc.tile_pool(name="stream", bufs=3) as spool,
            tc.tile_pool(name="work", bufs=2) as wpool,
            tc.tile_pool(name="tanh", bufs=8) as tpool,
            tc.tile_pool(name="psum", bufs=1, space=bass.MemorySpace.PSUM) as ppool,
        ):
            UaT = cpool.tile([128, 2, 2, H], f8, tag="UaT")
            va = cpool.tile([128, H], bf16, tag="va")
            vab = cpool.tile([128, 1], f32, tag="vab")
            cov = cpool.tile([128, NT, K], f32, tag="cov")
            Wst = cpool.tile([128, 2, 2, K * H], f8, tag="Wst")
            alph = cpool.tile([128, NT, K], f32, tag="alph")
            idt = cpool.tile([128, 128], bf16, tag="idt")

            # Small consts first so the first PK matmul unblocks fast;
            # Wstack is only needed once the first QW pass starts.
            # Two HWDGE rings: sync carries the matmul-critical stationaries
            # (UaT, topicsT, qT); scalar carries consts + epilogue traffic
            # (va, cov, Wstack, topicsN, outputs) so tile-0's stationaries
            # aren't queued behind megabytes of lower-priority transfers.
            nc.sync.dma_start(out=UaT[:], in_=UaT_d[:])
            nc.scalar.dma_start(out=va[:], in_=va_d[:])
            nc.scalar.dma_start(out=vab[:], in_=vab_d[:])
            nc.scalar.dma_start(out=cov[:], in_=cov_d[:])
            nc.scalar.dma_start(out=idt[:], in_=id_d[:])
            for k in range(K):
                nc.scalar.dma_start(
                    out=Wst[:, :, :, ts(k, 512)], in_=W_d[:, :, :, ts(k, 512)]
                )

            for t in range(NT):
                tT = spool.tile([128, K, 2, 2, 128], f8, tag="tT")
                nc.sync.dma_start(out=tT[:], in_=tT_d[t])
                qT = spool.tile([128, 2, 2, 128], f8, tag="qT")
                nc.sync.dma_start(out=qT[:], in_=qT_d[:, :, :, ts(t, 128)])

                ps = [
                    ppool.tile([128, 512], f32, tag=f"ps{k}", name=f"ps{k}")
                    for k in range(K)
                ]

                scores = wpool.tile([128, K], f32, tag="scores")
                # All PK matmuls first (need only tT+UaT — cheap DMA deps),
                # then QW k-outer so bank k completes staggered for tanh.
                for k in range(K):
                    for c2 in range(2):
                        nc.tensor.matmul(
                            ps[k][:],
                            tT[:, k, c2, :, :],
                            UaT[:, c2, :, :],
                            start=(c2 == 0),
                            stop=False,
                            perf_mode=DR,
                        )
                for k in range(K):
                    for c2 in range(2):
                        nc.tensor.matmul(
                            ps[k][:],
                            qT[:, c2, :, :],
                            Wst[:, c2, :, ts(k, 512)],
                            start=False,
                            stop=(c2 == 1),
                            perf_mode=DR,
                        )
                    th = tpool.tile([128, 512], bf16, tag="tanh")
                    nc.scalar.activation(th[:], ps[k][:], ACTF.Tanh)
                    scr = wpool.tile([128, 512], bf16, tag="ttr")
                    # scr = th * va; scores[:,k] = sum(scr)
                    nc.vector.scalar_tensor_tensor(
                        out=scr[:],
                        in0=th[:],
                        scalar=1.0,
                        in1=va[:],
                        op0=ALU.bypass,
                        op1=ALU.mult,
                        accum_out=scores[:, k : k + 1],
                    )

                # topics in natural layout — needed only by the mt epilogue,
                # so its DMA is emitted late (lower queue priority).
                tN = spool.tile([128, K, E], bf16, tag="tN")
                nc.scalar.dma_start(out=tN[:], in_=tN_d[t])

                # softmax over K: sc2 = (scores + va_b) * coverage
                sc2 = wpool.tile([128, K], f32, tag="sc2")
                nc.vector.scalar_tensor_tensor(
                    out=sc2[:],
                    in0=scores[:],
                    scalar=vab[:, 0:1],
                    in1=cov[:, t, :],
                    op0=ALU.add,
                    op1=ALU.mult,
                )
                negmax = wpool.tile([128, 1], f32, tag="negmax")
                nc.vector.reduce_max(negmax[:], sc2[:], axis=AX.X, negate=True)
                exps = wpool.tile([128, K], f32, tag="exps")
                nc.scalar.activation(
                    exps[:], sc2[:], ACTF.Exp, bias=negmax[:, 0:1], scale=1.0
                )
                ssum = wpool.tile([128, 1], f32, tag="ssum")
                nc.vector.reduce_sum(ssum[:], exps[:], axis=AX.X)
                rsum = wpool.tile([128, 1], f32, tag="rsum")
                nc.vector.reciprocal(rsum[:], ssum[:])
                nc.vector.tensor_scalar(
                    alph[:, t, :], exps[:], rsum[:, 0:1], None, ALU.mult
                )

                # mt = sum_k alpha_k * topics_k on the PE: stationary is
                # diag(alpha_k) (identity * per-partition alpha), fp32 PSUM
                # accumulation over k, single drain copy per tile.
                mtps = ppool.tile([128, 512], f32, tag="mtps", bufs=2)
                for k in range(K):
                    dg = wpool.tile(
                        [128, 128], bf16, tag="diag", bufs=6, name="dg"
                    )
                    nc.vector.tensor_scalar(
                        dg[:], idt[:], alph[:, t, k : k + 1], None, ALU.mult
                    )
                    nc.tensor.matmul(
                        mtps[:],
                        dg[:],
                        tN[:, k, :],
                        start=(k == 0),
                        stop=(k == K - 1),
                    )
                mt = wpool.tile([128, E], bf16, tag="mt")
                nc.vector.tensor_copy(mt[:], mtps[:])
                nc.scalar.dma_start(out=mt_d[t], in_=mt[:])

            nc.scalar.dma_start(out=al_d[:], in_=alph[:])

    nc.finalize()
    return nc


def _prep_core_inputs(query, topics, coverage_vector, Ua, Wa, va_w, va_b):
    """Build the 8 per-core input maps (all host-side layout/dtype work)."""
    # W_i = (Wa^T)^i, i=1..K, computed in float64 then cast once to bf16.
    WaT = Wa.astype(np.float64).T
    mats = []
    cur = np.eye(H, dtype=np.float64)
    for _ in range(K):
        cur = cur @ WaT
        mats.append(cur)
    Wstack = np.concatenate(mats, axis=1)  # (H, K*H)
    # DR layout [h_in, c2, i, col], h = c2*256 + i*128 + h_in
    W_sb = np.ascontiguousarray(
        Wstack.reshape(2, 2, 128, K * H).transpose(2, 0, 1, 3)
    ).astype(FP8)

    # DoubleRow fp8 layout for Ua^T: [e_in, pair(c2), row(i), h],
    # e = c2*256 + i*128 + e_in
    UaT_sb = np.ascontiguousarray(
        Ua.T.reshape(2, 2, 128, H).transpose(2, 0, 1, 3)
    ).astype(FP8)
    va_sb = np.ascontiguousarray(np.broadcast_to(va_w.astype(BF16), (128, H)))
    vab_sb = np.full((128, 1), np.float32(va_b[0]), np.float32)

    tb = topics.astype(BF16)          # (B, K, E)
    t8 = topics.astype(FP8)           # (B, K, E) for the PK matmul
    q8 = query.astype(FP8)            # (B, H) for the QW matmul
    cova = np.asarray(coverage_vector, np.float32)

    in_maps = []
    for c in range(NCORES):
        sl = slice(c * BS, (c + 1) * BS)
        tsh = tb[sl]                                       # (BS, K, E)
        tN = np.ascontiguousarray(tsh.reshape(NT, 128, K, E))
        # [t, e_in, k, c2, i, b] with e = c2*256 + i*128 + e_in
        tT = np.ascontiguousarray(
            t8[sl].reshape(NT, 128, K, 2, 2, 128).transpose(0, 5, 2, 3, 4, 1)
        )
        qT = np.ascontiguousarray(
            q8[sl].T.reshape(2, 2, 128, BS).transpose(2, 0, 1, 3)
        )
        cv = np.ascontiguousarray(cova[sl].reshape(NT, 128, K).transpose(1, 0, 2))
        in_maps.append(
            {
                "ident": np.eye(128, dtype=BF16),
                "qT": qT,
                "Wstack": W_sb,
                "UaT": UaT_sb,
                "topicsT": tT,
                "topicsN": tN,
                "cov": cv,
                "va": va_sb,
                "vab": vab_sb,
            }
        )
    return in_maps


def kernel(query, topics, coverage_vector, Ua, Wa, va_w, va_b):
    from concourse.bass_utils import run_bass_kernel_spmd

    query = np.asarray(query, np.float32)
    topics = np.asarray(topics, np.float32)
    coverage_vector = np.asarray(coverage_vector, np.float32)
    Ua = np.asarray(Ua, np.float32)
    Wa = np.asarray(Wa, np.float32)
    va_w = np.asarray(va_w, np.float32)
    va_b = np.asarray(va_b, np.float32)

    if "nc" not in _CACHE:
        _CACHE["nc"] = _build_nc()
    nc = _CACHE["nc"]

    in_maps = _prep_core_inputs(
        query, topics, coverage_vector, Ua, Wa, va_w, va_b
    )

    res = run_bass_kernel_spmd(
        nc, in_maps, core_ids=list(range(NCORES)), **RUN_KWARGS
    )
    LAST["exec_time_ns"] = res.exec_time_ns
    LAST["results"] = res

    mt = np.concatenate(
        [
            res.results[c]["mt"].astype(np.float32).reshape(BS, E)
            for c in range(NCORES)
        ],
        axis=0,
    )
    alphas = np.concatenate(
        [
            res.results[c]["alphas"].transpose(1, 0, 2).reshape(BS, K)
            for c in range(NCORES)
        ],
        axis=0,
    ).astype(np.float32)
    return (mt, alphas)


# revision 46
# speedup vs baseline: 1.8006x; 1.0363x over previous
"""Trainium2 Bass kernel for the coverage-attention module.

Computes, for B=16384, K=5, H=E=512:
    proj_keys = einsum("bke,he->bkh", topics, Ua)
    q_i = query @ (Wa^T)^i            (i = 1..K)
    s_i = tanh(q_i + proj_keys[:,i]) @ va_w^T + va_b
    scores[:, i] = s_i * coverage[:, i]
    alphas = softmax(scores, axis=1)
    mt = einsum("bk,bke->be", alphas, topics)
Returns (mt, alphas) like the reference.

Strategy: data-parallel over batch across 8 NeuronCores (2048 rows/core).
Host precomputes W_i = (Wa^T)^i in float64 and ships [W_1..W_K] so the
query chain becomes one matmul; both matmuls accumulate into the same
PSUM bank so the q+proj_key add is free.  All matmul operands are bf16
(host-cast); accumulation and the softmax/weighted-sum epilogue are fp32.
"""

import os
import sys

import numpy as np

try:
    import ml_dtypes
except ImportError:  # pragma: no cover
    sys.path.insert(0, "/opt/trn_rl_repo")
    import ml_dtypes

for _p in ("/opt/trn_rl_repo",):
    if _p not in sys.path and os.path.isdir(_p):
        sys.path.insert(0, _p)

BF16 = ml_dtypes.bfloat16
FP8 = ml_dtypes.float8_e4m3

B, K, H, E = 16384, 5, 512, 512
NCORES = 8
BS = B // NCORES          # rows per core = 2048
NT = BS // 128            # 128-row tiles per core = 16
HC = H // 128             # contraction chunks over h = 4
EC = E // 128             # contraction chunks over e = 4

_CACHE = {}
LAST = {}                 # exec_time_ns etc. from the most recent run
RUN_KWARGS = {}           # extra kwargs for run_bass_kernel_spmd (test harness)


def _build_nc():
    import concourse.bass as bass
    import concourse.mybir as mybir
    import concourse.tile as tile
    from concourse import bacc
    from concourse.bass import ts

    f32 = mybir.dt.float32
    bf16 = mybir.dt.bfloat16
    f8 = mybir.dt.float8e4
    DR = mybir.MatmulPerfMode.DoubleRow
    ALU = mybir.AluOpType
    ACTF = mybir.ActivationFunctionType
    AX = mybir.AxisListType

    nc = bacc.Bacc()

    qT_d = nc.declare_dram_parameter("qT", [128, 2, 2, BS], f8, isOutput=False)
    W_d = nc.declare_dram_parameter("Wstack", [128, 2, 2, K * H], f8, isOutput=False)
    # fp8 DoubleRow operands: [.., 2(pair), 2(row-in-pair), ..]
    UaT_d = nc.declare_dram_parameter("UaT", [128, 2, 2, H], f8, isOutput=False)
    tT_d = nc.declare_dram_parameter(
        "topicsT", [NT, 128, K, 2, 2, 128], f8, isOutput=False
    )
    tN_d = nc.declare_dram_parameter("topicsN", [NT, 128, K, E], bf16, isOutput=False)
    cov_d = nc.declare_dram_parameter("cov", [128, NT, K], f32, isOutput=False)
    va_d = nc.declare_dram_parameter("va", [128, H], bf16, isOutput=False)
    vab_d = nc.declare_dram_parameter("vab", [128, 1], f32, isOutput=False)
    id_d = nc.declare_dram_parameter("ident", [128, 128], bf16, isOutput=False)

    mt_d = nc.declare_dram_parameter("mt", [NT, 128, E], bf16, isOutput=True)
    al_d = nc.declare_dram_parameter("alphas", [128, NT, K], f32, isOutput=True)

    with tile.TileContext(nc) as tc:
        with (
            tc.tile_pool(name="const", bufs=1) as cpool,
            tc.tile_pool(name="stream", bufs=3) as spool,
            tc.tile_pool(name="work", bufs=2) as wpool,
            tc.tile_pool(name="tanh", bufs=8) as tpool,
            tc.tile_pool(name="psum", bufs=1, space=bass.MemorySpace.PSUM) as ppool,
        ):
            UaT = cpool.tile([128, 2, 2, H], f8, tag="UaT")
            va = cpool.tile([128, H], bf16, tag="va")
            vab = cpool.tile([128, 1], f32, tag="vab")
            cov = cpool.tile([128, NT, K], f32, tag="cov")
            Wst = cpool.tile([128, 2, 2, K * H], f8, tag="Wst")
            alph = cpool.tile([128, NT, K], f32, tag="alph")
            idt = cpool.tile([128, 128], bf16, tag="idt")

            # Small consts first so the first PK matmul unblocks fast;
            # Wstack is only needed once the first QW pass starts.
            # Two HWDGE rings: sync carries the matmul-critical stationaries
            # (UaT, topicsT, qT); scalar carries consts + epilogue traffic
            # (va, cov, Wstack, topicsN, outputs) so tile-0's stationaries
            # aren't queued behind megabytes of lower-priority transfers.
            nc.sync.dma_start(out=UaT[:], in_=UaT_d[:])
            nc.scalar.dma_start(
                out=Wst[:, :, :, ts(0, 512)], in_=W_d[:, :, :, ts(0, 512)]
            )
            nc.scalar.dma_start(out=va[:], in_=va_d[:])
            nc.scalar.dma_start(out=vab[:], in_=vab_d[:])
            nc.scalar.dma_start(out=cov[:], in_=cov_d[:])
            nc.scalar.dma_start(out=idt[:], in_=id_d[:])
            for k in range(1, K):
                nc.scalar.dma_start(
                    out=Wst[:, :, :, ts(k, 512)], in_=W_d[:, :, :, ts(k, 512)]
                )

            for t in range(NT):
                tT = spool.tile([128, K, 2, 2, 128], f8, tag="tT")
                nc.sync.dma_start(out=tT[:], in_=tT_d[t])
                qT = spool.tile([128, 2, 2, 128], f8, tag="qT")
                nc.sync.dma_start(out=qT[:], in_=qT_d[:, :, :, ts(t, 128)])

                ps = [
                    ppool.tile([128, 512], f32, tag=f"ps{k}", name=f"ps{k}")
                    for k in range(K)
                ]

                scores = wpool.tile([128, K], f32, tag="scores")
                # All PK matmuls first (need only tT+UaT — cheap DMA deps),
                # then QW k-outer so bank k completes staggered for tanh.
                for k in range(K):
                    for c2 in range(2):
                        nc.tensor.matmul(
                            ps[k][:],
                            tT[:, k, c2, :, :],
                            UaT[:, c2, :, :],
                            start=(c2 == 0),
                            stop=False,
                            perf_mode=DR,
                        )
                for k in range(K):
                    for c2 in range(2):
                        nc.tensor.matmul(
                            ps[k][:],
                            qT[:, c2, :, :],
                            Wst[:, c2, :, ts(k, 512)],
                            start=False,
                            stop=(c2 == 1),
                            perf_mode=DR,
                        )
                    th = tpool.tile([128, 512], bf16, tag="tanh")
                    nc.scalar.activation(th[:], ps[k][:], ACTF.Tanh)
                    scr = wpool.tile([128, 512], bf16, tag="ttr")
                    # scr = th * va; scores[:,k] = sum(scr)
                    nc.vector.scalar_tensor_tensor(
                        out=scr[:],
                        in0=th[:],
                        scalar=1.0,
                        in1=va[:],
                        op0=ALU.bypass,
                        op1=ALU.mult,
                        accum_out=scores[:, k : k + 1],
                    )

                # topics in natural layout — needed only by the mt epilogue,
                # so its DMA is emitted late (lower queue priority).
                tN = spool.tile([128, K, E], bf16, tag="tN")
                nc.scalar.dma_start(out=tN[:], in_=tN_d[t])

                # softmax over K: sc2 = (scores + va_b) * coverage
                sc2 = wpool.tile([128, K], f32, tag="sc2")
                nc.vector.scalar_tensor_tensor(
                    out=sc2[:],
                    in0=scores[:],
                    scalar=vab[:, 0:1],
                    in1=cov[:, t, :],
                    op0=ALU.add,
                    op1=ALU.mult,
                )
                negmax = wpool.tile([128, 1], f32, tag="negmax")
                nc.vector.reduce_max(negmax[:], sc2[:], axis=AX.X, negate=True)
                exps = wpool.tile([128, K], f32, tag="exps")
                nc.scalar.activation(
                    exps[:], sc2[:], ACTF.Exp, bias=negmax[:, 0:1], scale=1.0
                )
                ssum = wpool.tile([128, 1], f32, tag="ssum")
                nc.vector.reduce_sum(ssum[:], exps[:], axis=AX.X)
                rsum = wpool.tile([128, 1], f32, tag="rsum")
                nc.vector.reciprocal(rsum[:], ssum[:])
                nc.vector.tensor_scalar(
                    alph[:, t, :], exps[:], rsum[:, 0:1], None, ALU.mult
                )

                # mt = sum_k alpha_k * topics_k on the PE: stationary is
                # diag(alpha_k) (identity * per-partition alpha), fp32 PSUM
                # accumulation over k, single drain copy per tile.
                mtps = ppool.tile([128, 512], f32, tag="mtps", bufs=2)
                for k in range(K):
                    dg = wpool.tile(
                        [128, 128], bf16, tag="diag", bufs=6, name="dg"
                    )
                    nc.vector.tensor_scalar(
                        dg[:], idt[:], alph[:, t, k : k + 1], None, ALU.mult
                    )
                    nc.tensor.matmul(
                        mtps[:],
                        dg[:],
                        tN[:, k, :],
                        start=(k == 0),
                        stop=(k == K - 1),
                    )
                mt = wpool.tile([128, E], bf16, tag="mt")
                nc.scalar.activation(mt[:], mtps[:], ACTF.Copy)
                nc.sync.dma_start(out=mt_d[t], in_=mt[:])

            nc.scalar.dma_start(out=al_d[:], in_=alph[:])

    nc.finalize()
    return nc


def _prep_core_inputs(query, topics, coverage_vector, Ua, Wa, va_w, va_b):
    """Build the 8 per-core input maps (all host-side layout/dtype work)."""
    # W_i = (Wa^T)^i, i=1..K, computed in float64 then cast once to bf16.
    WaT = Wa.astype(np.float64).T
    mats = []
    cur = np.eye(H, dtype=np.float64)
    for _ in range(K):
        cur = cur @ WaT
        mats.append(cur)
    Wstack = np.concatenate(mats, axis=1)  # (H, K*H)
    # DR layout [h_in, c2, i, col], h = c2*256 + i*128 + h_in
    W_sb = np.ascontiguousarray(
        Wstack.reshape(2, 2, 128, K * H).transpose(2, 0, 1, 3)
    ).astype(FP8)

    # DoubleRow fp8 layout for Ua^T: [e_in, pair(c2), row(i), h],
    # e = c2*256 + i*128 + e_in
    UaT_sb = np.ascontiguousarray(
        Ua.T.reshape(2, 2, 128, H).transpose(2, 0, 1, 3)
    ).astype(FP8)
    va_sb = np.ascontiguousarray(np.broadcast_to(va_w.astype(BF16), (128, H)))
    vab_sb = np.full((128, 1), np.float32(va_b[0]), np.float32)

    tb = topics.astype(BF16)          # (B, K, E)
    t8 = topics.astype(FP8)           # (B, K, E) for the PK matmul
    q8 = query.astype(FP8)            # (B, H) for the QW matmul
    cova = np.asarray(coverage_vector, np.float32)

    in_maps = []
    for c in range(NCORES):
        sl = slice(c * BS, (c + 1) * BS)
        tsh = tb[sl]                                       # (BS, K, E)
        tN = np.ascontiguousarray(tsh.reshape(NT, 128, K, E))
        # [t, e_in, k, c2, i, b] with e = c2*256 + i*128 + e_in
        tT = np.ascontiguousarray(
            t8[sl].reshape(NT, 128, K, 2, 2, 128).transpose(0, 5, 2, 3, 4, 1)
        )
        qT = np.ascontiguousarray(
            q8[sl].T.reshape(2, 2, 128, BS).transpose(2, 0, 1, 3)
        )
        cv = np.ascontiguousarray(cova[sl].reshape(NT, 128, K).transpose(1, 0, 2))
        in_maps.append(
            {
                "ident": np.eye(128, dtype=BF16),
                "qT": qT,
                "Wstack": W_sb,
                "UaT": UaT_sb,
                "topicsT": tT,
                "topicsN": tN,
                "cov": cv,
                "va": va_sb,
                "vab": vab_sb,
            }
        )
    return in_maps


def kernel(query, topics, coverage_vector, Ua, Wa, va_w, va_b):
    from concourse.bass_utils import run_bass_kernel_spmd

    query = np.asarray(query, np.float32)
    topics = np.asarray(topics, np.float32)
    coverage_vector = np.asarray(coverage_vector, np.float32)
    Ua = np.asarray(Ua, np.float32)
    Wa = np.asarray(Wa, np.float32)
    va_w = np.asarray(va_w, np.float32)
    va_b = np.asarray(va_b, np.float32)

    if "nc" not in _CACHE:
        _CACHE["nc"] = _build_nc()
    nc = _CACHE["nc"]

    in_maps = _prep_core_inputs(
        query, topics, coverage_vector, Ua, Wa, va_w, va_b
    )

    res = run_bass_kernel_spmd(
        nc, in_maps, core_ids=list(range(NCORES)), **RUN_KWARGS
    )
    LAST["exec_time_ns"] = res.exec_time_ns
    LAST["results"] = res

    mt = np.concatenate(
        [
            res.results[c]["mt"].astype(np.float32).reshape(BS, E)
            for c in range(NCORES)
        ],
        axis=0,
    )
    alphas = np.concatenate(
        [
            res.results[c]["alphas"].transpose(1, 0, 2).reshape(BS, K)
            for c in range(NCORES)
        ],
        axis=0,
    ).astype(np.float32)
    return (mt, alphas)
